# revision 4
# baseline (speedup 1.0000x reference)
"""Trainium2 Bass kernel for nn_MultiHeadDuelingDQN (8-core SPMD).

Model (B=256, STATE=26240, H=512, R=4000, N=64 heads, M=10):
    h  = relu(relu(x@W1+b1)@W2+b2)
    q_cache = h@Wvc+bvc + (h@Wac+bac) - mean_R(h@Wac+bac)
    q_assoc = per-head dueling over M (local means)
    q_rec   = S - mean_R(S),  S = sum_n (h@Wru[n]+bru[n])   [exact rewrite:
              rec_global has zero row-mean, so the reference's second mean
              subtraction is a no-op and S never needs the [B,N,R] tensor]

Sharding (8 cores):
  - fc1: contraction (STATE) split 8 ways -> AllReduce of h1_pre [512,256]
  - fc2: replicated
  - rec/cache: R split 8 ways (500 cols/core); the sum over heads becomes a
    DVE pre-sum of Wru tiles (W_sum = sum_n Wru[n,:,rslice]) streamed from
    HBM, then one small matmul h @ W_sum.  Row-means over the full R need a
    tiny [128,4] AllReduce of partial row-sums.
  - assoc heads: split 8 ways (8 heads/core), fully local; augmented matmul
    [Wau | Wvu | Wvc] -> [adv_assoc | val_n | value_c] in one pass.

kernel(**inputs) takes full unsharded inputs, returns full [256, 8640].
"""
import numpy as np

import concourse.bass as bass
import concourse.mybir as mybir
import concourse.tile as tile
from concourse import bacc
from concourse import bass_utils
from concourse.bass import ts
from concourse.masks import make_identity

NC = 8
B, H, STATE, R, NH, M = 256, 512, 26240, 4000, 64, 10
KPC_RAW = STATE // NC          # 3280
KCH = 26                       # k-chunks of 128 per core (padded)
KPC = KCH * 128                # 3328
RPC = R // NC                  # 500
HPC = NH // NC                 # 8 heads per core
AUG = HPC * (M + 1) + 1        # 89 = [8x(10 adv + 1 val)] + value_c
F32 = mybir.dt.float32
RELU = mybir.ActivationFunctionType.Relu
COPY = mybir.ActivationFunctionType.Copy
ADD = mybir.AluOpType.add
SUB = mybir.AluOpType.subtract


def build_program(wru_bufs=16):
    nc = bacc.Bacc("TRN2", target_bir_lowering=False, debug=False, num_devices=NC)

    # ---- per-core I/O ----
    xs = nc.dram_tensor("xs", [B, KPC], F32, kind="ExternalInput").ap()
    w1s = nc.dram_tensor("w1s", [KPC, H], F32, kind="ExternalInput").ap()
    b1 = nc.dram_tensor("b1", [H], F32, kind="ExternalInput").ap()
    w2 = nc.dram_tensor("w2", [H, H], F32, kind="ExternalInput").ap()
    b2 = nc.dram_tensor("b2", [H], F32, kind="ExternalInput").ap()
    wac = nc.dram_tensor("wac", [H, RPC], F32, kind="ExternalInput").ap()
    bac = nc.dram_tensor("bac", [RPC], F32, kind="ExternalInput").ap()
    wru = nc.dram_tensor("wru", [NH, H, RPC], F32, kind="ExternalInput").ap()
    bru = nc.dram_tensor("bru", [NH, RPC], F32, kind="ExternalInput").ap()
    wau = nc.dram_tensor("wau", [HPC, H, M], F32, kind="ExternalInput").ap()
    bau = nc.dram_tensor("bau", [HPC, M], F32, kind="ExternalInput").ap()
    wvu = nc.dram_tensor("wvu", [HPC, H], F32, kind="ExternalInput").ap()
    bvu = nc.dram_tensor("bvu", [HPC], F32, kind="ExternalInput").ap()
    wvc = nc.dram_tensor("wvc", [H], F32, kind="ExternalInput").ap()
    bvc = nc.dram_tensor("bvc", [1], F32, kind="ExternalInput").ap()

    out_cache = nc.dram_tensor("out_cache", [B, RPC], F32, kind="ExternalOutput").ap()
    out_rec = nc.dram_tensor("out_rec", [B, RPC], F32, kind="ExternalOutput").ap()
    out_assoc = nc.dram_tensor("out_assoc", [B, HPC * M], F32, kind="ExternalOutput").ap()

    with tile.TileContext(nc) as tc:
        with (
            tc.tile_pool(name="cst", bufs=1) as cst,
            tc.tile_pool(name="sb", bufs=1) as sb,
            tc.tile_pool(name="w1p", bufs=6) as w1p,
            tc.tile_pool(name="wrup", bufs=wru_bufs) as wrup,
            tc.tile_pool(name="ps", bufs=2, space="PSUM") as ps,
            tc.tile_pool(name="psfc", bufs=4, space="PSUM") as psfc,
            tc.tile_pool(name="dram", bufs=1, space="DRAM") as dram,
        ):
            ident = cst.tile([128, 128], F32, tag="ident")
            make_identity(nc, ident)
            ones1 = cst.tile([1, 128], F32, tag="ones1")
            nc.vector.memset(ones1, 1.0)
            ones64 = cst.tile([64, 128], F32, tag="ones64")
            nc.vector.memset(ones64, 1.0)

            # ---------- Phase A: trunk ----------
            # transpose x slice: [256, 3328] -> xT chunks [128(k), 256(b)]
            x_sb = []
            for bt in range(2):
                t = sb.tile([128, KPC], F32, tag=f"x_sb{bt}")
                nc.sync.dma_start(t, xs[ts(bt, 128), :])
                x_sb.append(t)
            xT = []
            for kc in range(KCH):
                t = sb.tile([128, B], F32, tag=f"xT{kc}")
                for bt in range(2):
                    pt = ps.tile([128, 128], F32, tag="small")
                    nc.tensor.transpose(pt, x_sb[bt][:, ts(kc, 128)], ident)
                    nc.vector.tensor_copy(t[:, ts(bt, 128)], pt)
                xT.append(t)

            # fc1 partial: h1_ps[m] [128(h1), 256(b)] += W1[kc,m].T @ xT[kc]
            h1_ps = [psfc.tile([128, B], F32, tag="fc", name=f"h1_ps{i}")
                     for i in range(4)]
            for kc in range(KCH):
                w1t = w1p.tile([128, H], F32, tag="w1")
                nc.sync.dma_start(w1t, w1s[ts(kc, 128), :])
                for m in range(4):
                    nc.tensor.matmul(h1_ps[m], w1t[:, ts(m, 128)], xT[kc],
                                     start=(kc == 0), stop=(kc == KCH - 1))

            # AllReduce h1_pre over 8 cores
            ar1_in = dram.tile([H, B], F32, tag="ar1_in")
            ar1_out = dram.tile([H, B], F32, tag="ar1_out")
            for m in range(4):
                t = sb.tile([128, B], F32, tag=f"h1c{m}")
                nc.vector.tensor_copy(t, h1_ps[m])
                nc.sync.dma_start(ar1_in[ts(m, 128), :], t)
            nc.gpsimd.collective_compute(
                "AllReduce", ADD, replica_groups=[list(range(NC))],
                ins=[ar1_in.opt()], outs=[ar1_out.opt()],
            )

            # h1T[m] = relu(h1_pre + b1)
            h1T = []
            for m in range(4):
                bt1 = cst.tile([128, 1], F32, tag=f"b1t{m}")
                nc.sync.dma_start(bt1, b1[ts(m, 128)].rearrange("(a b) -> a b", b=1))
                raw = sb.tile([128, B], F32, tag=f"h1raw{m}")
                nc.sync.dma_start(raw, ar1_out[ts(m, 128), :])
                t = sb.tile([128, B], F32, tag=f"h1T{m}")
                nc.scalar.activation(t, raw, RELU, bias=bt1, scale=1.0)
                h1T.append(t)

            # fc2: hT[m2] = relu(sum_kc W2[kc,m2].T @ h1T[kc] + b2)
            w2t = []
            for kc in range(4):
                t = sb.tile([128, H], F32, tag=f"w2_{kc}")
                nc.sync.dma_start(t, w2[ts(kc, 128), :])
                w2t.append(t)
            hT = []
            for m2 in range(4):
                acc2 = psfc.tile([128, B], F32, tag="fc")
                for kc in range(4):
                    nc.tensor.matmul(acc2, w2t[kc][:, ts(m2, 128)], h1T[kc],
                                     start=(kc == 0), stop=(kc == 3))
                bt2 = cst.tile([128, 1], F32, tag=f"b2t{m2}")
                nc.sync.dma_start(bt2, b2[ts(m2, 128)].rearrange("(a b) -> a b", b=1))
                t = sb.tile([128, B], F32, tag=f"hT{m2}")
                nc.scalar.activation(t, acc2, RELU, bias=bt2, scale=1.0)
                hT.append(t)

            # ---------- Phase B: assoc heads (augmented [adv|val|value_c]) ----------
            aug_w = []
            for kc in range(4):
                t = cst.tile([128, AUG], F32, tag=f"aug_w{kc}")
                grid = t[:, 0:HPC * (M + 1)].rearrange("p (n u) -> p n u", u=M + 1)
                nc.sync.dma_start(
                    grid[:, :, 0:M],
                    wau[:, ts(kc, 128), :].rearrange("n k m -> k n m"))
                nc.sync.dma_start(
                    grid[:, :, M:M + 1],
                    wvu[:, ts(kc, 128)].rearrange("n (k u) -> k n u", u=1))
                nc.sync.dma_start(
                    t[:, AUG - 1:AUG],
                    wvc[ts(kc, 128)].rearrange("(k u) -> k u", u=1))
                aug_w.append(t)
            aug_b = cst.tile([1, AUG], F32, tag="aug_b")
            bgrid = aug_b[:, 0:HPC * (M + 1)].rearrange("p (n u) -> p n u", u=M + 1)
            nc.sync.dma_start(bgrid[:, :, 0:M], bau.rearrange("n (a m) -> a n m", a=1))
            nc.sync.dma_start(bgrid[:, :, M:M + 1],
                              bvu.rearrange("(a n u) -> a n u", a=1, u=1))
            nc.sync.dma_start(aug_b[:, AUG - 1:AUG], bvc.rearrange("(a u) -> a u", a=1))

            value_sb = []
            for bt in range(2):
                psA = ps.tile([128, AUG], F32, tag="small")
                for kc in range(4):
                    nc.tensor.matmul(psA, hT[kc][:, ts(bt, 128)], aug_w[kc],
                                     start=(kc == 0), stop=False)
                nc.tensor.matmul(psA, ones1, aug_b, start=False, stop=True)
                adv = psA[:, 0:HPC * (M + 1)].rearrange("p (n u) -> p n u", u=M + 1)
                sumA = sb.tile([128, HPC], F32, tag=f"sumA{bt}")
                nc.vector.tensor_reduce(sumA, adv[:, :, 0:M],
                                        axis=mybir.AxisListType.X, op=ADD)
                scaled = sb.tile([128, HPC], F32, tag=f"scaledA{bt}")
                nc.scalar.activation(scaled, sumA, COPY, scale=1.0 / M)
                tmp = sb.tile([128, HPC], F32, tag=f"tmpA{bt}")
                nc.vector.tensor_tensor(out=tmp, in0=adv[:, :, M], in1=scaled, op=SUB)
                q = sb.tile([128, HPC * M], F32, tag=f"qA{bt}")
                nc.vector.tensor_tensor(
                    out=q.rearrange("p (n m) -> p n m", m=M),
                    in0=adv[:, :, 0:M],
                    in1=tmp.broadcast_to([128, HPC, M]),
                    op=ADD)
                nc.sync.dma_start(out_assoc[ts(bt, 128), :], q)
                v = sb.tile([128, 1], F32, tag=f"valc{bt}")
                nc.vector.tensor_copy(v, psA[:, AUG - 1:AUG])
                value_sb.append(v)

            # ---------- Phase C: cache head (R-slice) ----------
            ar2_in = sb.tile([128, 4], F32, tag="ar2_in")
            wac_t = []
            for kc in range(4):
                t = sb.tile([128, RPC], F32, tag=f"wac{kc}")
                nc.sync.dma_start(t, wac[ts(kc, 128), :])
                wac_t.append(t)
            bac_sb = cst.tile([1, RPC], F32, tag="bac_sb")
            nc.sync.dma_start(bac_sb, bac.rearrange("(a r) -> a r", a=1))
            adv_c_sb = []
            for bt in range(2):
                psC = ps.tile([128, RPC], F32, tag="wide")
                for kc in range(4):
                    nc.tensor.matmul(psC, hT[kc][:, ts(bt, 128)], wac_t[kc],
                                     start=(kc == 0), stop=False)
                nc.tensor.matmul(psC, ones1, bac_sb, start=False, stop=True)
                t = sb.tile([128, RPC], F32, tag=f"advc{bt}")
                nc.vector.tensor_copy(t, psC)
                adv_c_sb.append(t)
                nc.vector.tensor_reduce(ar2_in[:, bt:bt + 1], t,
                                        axis=mybir.AxisListType.X, op=ADD)

            # ---------- Phase D: Wru stream + head pre-sum (DVE) ----------
            acc = [sb.tile([128, RPC], F32, tag=f"acc{k}", name=f"acc{k}")
                   for k in range(4)]
            for kc in range(4):
                for n in range(NH):
                    wt = wrup.tile([128, RPC], F32, tag="wru")
                    nc.sync.dma_start(wt, wru[n, ts(kc, 128), :])
                    if n == 0:
                        nc.vector.tensor_copy(acc[kc], wt)
                    else:
                        nc.vector.tensor_add(acc[kc], acc[kc], wt)
            bru_sb = sb.tile([64, RPC], F32, tag="bru_sb")
            nc.sync.dma_start(bru_sb, bru)

            # S = hT.T @ W_sum (+ sum_n bru fold), then partial row-sums
            psS = []
            for bt in range(2):
                t = ps.tile([128, RPC], F32, tag="wide")
                for kc in range(4):
                    nc.tensor.matmul(t, hT[kc][:, ts(bt, 128)], acc[kc],
                                     start=(kc == 0), stop=False)
                nc.tensor.matmul(t, ones64, bru_sb, start=False, stop=True)
                nc.vector.tensor_reduce(ar2_in[:, 2 + bt:3 + bt], t,
                                        axis=mybir.AxisListType.X, op=ADD)
                psS.append(t)

            # ---------- Phase E: tiny AllReduce of row-sums, finalize ----------
            ar2_din = dram.tile([128, 4], F32, tag="ar2_din")
            ar2_dout = dram.tile([128, 4], F32, tag="ar2_dout")
            nc.sync.dma_start(ar2_din, ar2_in)
            nc.gpsimd.collective_compute(
                "AllReduce", ADD, replica_groups=[list(range(NC))],
                ins=[ar2_din.opt()], outs=[ar2_dout.opt()],
            )
            ar2_sb = sb.tile([128, 4], F32, tag="ar2_sb")
            nc.sync.dma_start(ar2_sb, ar2_dout)
            means = sb.tile([128, 4], F32, tag="means")
            nc.scalar.activation(means, ar2_sb, COPY, scale=1.0 / R)

            for bt in range(2):
                vm = sb.tile([128, 1], F32, tag=f"vm{bt}")
                nc.vector.tensor_tensor(out=vm, in0=value_sb[bt],
                                        in1=means[:, bt:bt + 1], op=SUB)
                qc = sb.tile([128, RPC], F32, tag=f"qc{bt}")
                nc.vector.tensor_scalar(out=qc, in0=adv_c_sb[bt], scalar1=vm,
                                        scalar2=None, op0=ADD)
                nc.sync.dma_start(out_cache[ts(bt, 128), :], qc)

                qr = sb.tile([128, RPC], F32, tag=f"qr{bt}")
                nc.vector.tensor_scalar(out=qr, in0=psS[bt],
                                        scalar1=means[:, 2 + bt:3 + bt],
                                        scalar2=None, op0=SUB)
                nc.sync.dma_start(out_rec[ts(bt, 128), :], qr)

    nc.compile()
    return nc


_CACHED = None


def _get_program():
    global _CACHED
    if _CACHED is None:
        _CACHED = build_program()
    return _CACHED


def make_in_maps(x, W1, b1, W2, b2, Wvc, bvc, Wac, bac, Wvu, bvu, Wau, bau, Wru, bru):
    f = np.float32
    in_maps = []
    for c in range(NC):
        k0 = c * KPC_RAW
        xs = np.zeros((B, KPC), f)
        xs[:, :KPC_RAW] = x[:, k0:k0 + KPC_RAW]
        w1s = np.zeros((KPC, H), f)
        w1s[:KPC_RAW] = W1[k0:k0 + KPC_RAW]
        r0 = c * RPC
        h0 = c * HPC
        in_maps.append({
            "xs": xs, "w1s": w1s,
            "b1": np.asarray(b1, f), "w2": np.asarray(W2, f), "b2": np.asarray(b2, f),
            "wac": np.ascontiguousarray(np.asarray(Wac, f)[:, r0:r0 + RPC]),
            "bac": np.ascontiguousarray(np.asarray(bac, f)[r0:r0 + RPC]),
            "wru": np.ascontiguousarray(np.asarray(Wru, f)[:, :, r0:r0 + RPC]),
            "bru": np.ascontiguousarray(np.asarray(bru, f)[:, r0:r0 + RPC]),
            "wau": np.ascontiguousarray(np.asarray(Wau, f)[h0:h0 + HPC]),
            "bau": np.ascontiguousarray(np.asarray(bau, f)[h0:h0 + HPC]),
            "wvu": np.ascontiguousarray(np.asarray(Wvu, f)[h0:h0 + HPC]),
            "bvu": np.ascontiguousarray(np.asarray(bvu, f)[h0:h0 + HPC]),
            "wvc": np.ascontiguousarray(np.asarray(Wvc, f).reshape(H)),
            "bvc": np.asarray(bvc, f).reshape(1),
        })
    return in_maps


def assemble(results):
    q = np.empty((B, 2 * R + NH * M), np.float32)
    for c in range(NC):
        r0 = c * RPC
        a0 = c * HPC * M
        q[:, r0:r0 + RPC] = results[c]["out_cache"]
        q[:, R + r0:R + r0 + RPC] = results[c]["out_rec"]
        q[:, 2 * R + a0:2 * R + a0 + HPC * M] = results[c]["out_assoc"]
    return q


def run(in_maps, **kw):
    nc = _get_program()
    return bass_utils.run_bass_kernel_spmd(nc, in_maps, core_ids=list(range(NC)), **kw)


def kernel(**inputs):
    in_maps = make_in_maps(**{k: np.asarray(v) for k, v in inputs.items()})
    res = run(in_maps)
    return assemble(res.results)
